# revision 30
# baseline (speedup 1.0000x reference)
"""Trainium2 Bass kernel for a 2-layer GAT (nn_GAT_70909910057105).

Strategy (8 NeuronCores, SPMD):
  - Core k owns target nodes [128k, 128k+128). Edges bucketed by trg//128 on
    the host (integer/layout-only preprocessing), padded to a uniform E_pad.
  - Per-edge edge_feature rows are staged host-side (pure row selection) as a
    transposed f32 input efT [C, E_pad], so pe = efT.T @ wesum is a plain
    matmul burst on device - no edge-feature gather, no PE transposes. The
    burst runs while the CC barrier + table-1 AllGather complete.
  - Node tables ([1024 bf16 h (b,d,h layout) | 16 f32 a_src | pad] rows) are
    exchanged with one AllGather per layer (a tiny warm-up collective absorbs
    the CC cold-start) and fetched per-edge with dma_gather on 4 SWDGE queues.
  - segment_sum is a PSUM-accumulated bf16 matmul with host-built one-hot
    masks; per-edge target alphas come from software-pipelined maskT @ at
    matmuls, summed with pe before the gather lands.
  - The h row layout (b, d, h) (host-permuted Wn columns) keeps the exp*h
    multiply inner-contiguous so the DVE runs it dual-pumped.
"""
import sys

for _p in ("/opt/trn_rl_repo", "/root/.axon_site/_ro/trn_rl_repo"):
    if _p not in sys.path:
        sys.path.insert(0, _p)

import numpy as np
import ml_dtypes
import concourse.bass as bass
import concourse.bacc as bacc
import concourse.tile as tile
from concourse import mybir
from concourse.bass_utils import run_bass_kernel_spmd
from concourse.masks import make_identity

F32 = mybir.dt.float32
BF16 = mybir.dt.bfloat16
I16 = mybir.dt.int16
NPBF = ml_dtypes.bfloat16

N, B, C, H, D = 1024, 4, 256, 4, 64
E = 32768
NC = 8
TPC = N // NC           # target nodes per core = 128
ROW = 1152              # bf16 elems: 1024 h | 32 (16 f32 a_src) | 96 pad
AS_OFF = 1024           # bf16-elem offset of the a_src f32 region
NB_LOCAL = TPC * B      # 512 local (node, batch) columns
WA = C + 2 * H          # fused table rhs width: wncols ++ a_sb

# permutation: table h column j = d*4 + h  <->  model channel h*64 + d
_PERM = np.array([(j % H) * D + j // H for j in range(C)])


# --------------------------------------------------------------------------
# host-side preprocessing (integer / layout ops only)
# --------------------------------------------------------------------------

def _pack_idx(vals: np.ndarray) -> np.ndarray:
    n = vals.shape[0]
    assert n % 16 == 0
    blk = vals.astype(np.int16).reshape(n // 16, 16).T
    return np.ascontiguousarray(np.tile(blk, (8, 1)))


def _prep(x, edge_features, src_idx, trg_idx,
          Wn1, We1, a_src1, a_tgt1, a_edge1,
          Wn2, We2, a_src2, a_tgt2, a_edge2):
    src = np.asarray(src_idx).astype(np.int64)
    trg = np.asarray(trg_idx).astype(np.int64)
    x = np.asarray(x, dtype=np.float32)
    ef = np.asarray(edge_features, dtype=np.float32)

    eids_per_core = [np.nonzero((trg // TPC) == k)[0] for k in range(NC)]
    E_pad = ((max(len(e) for e in eids_per_core) + 127) // 128) * 128

    xf = x.reshape(N * B, C)
    xT = np.ascontiguousarray(xf.T).astype(NPBF)

    def sb3(w, inner):
        return np.ascontiguousarray(w.reshape(2, 128, inner).transpose(1, 0, 2))

    def hsel(a_e):
        m = np.zeros((C, H), np.float32)
        for h in range(H):
            m[h * D:(h + 1) * D, h] = np.float32(a_e[h])
        return sb3(m, H)

    def ablk(a_s, a_t):
        m = np.zeros((C, 2 * H), np.float32)
        for h in range(H):
            m[h * D:(h + 1) * D, h] = np.asarray(a_s)[h]
            m[h * D:(h + 1) * D, H + h] = np.asarray(a_t)[h]
        return sb3(m, 2 * H)

    def wncols_p(Wn):
        wt = np.ascontiguousarray(np.asarray(Wn, np.float32).T)  # [c, c']
        return sb3(wt[:, _PERM].astype(NPBF), C)

    common = {
        "xTf": np.ascontiguousarray(xT).reshape(2, 128, N * B)
                 .transpose(1, 0, 2).copy(),
        "wn1cols": wncols_p(Wn1),
        "wn2cols": wncols_p(Wn2),
        "wn1hd": sb3(np.asarray(Wn1, np.float32), C),
        "wn2hd": sb3(np.asarray(Wn2, np.float32), C),
        "we1hd": sb3(np.asarray(We1, np.float32), C),
        "we2hd": sb3(np.asarray(We2, np.float32), C),
        "hsel1": hsel(np.asarray(a_edge1)),
        "hsel2": hsel(np.asarray(a_edge2)),
        "ablk1": ablk(a_src1, a_tgt1),
        "ablk2": ablk(a_src2, a_tgt2),
    }

    in_maps = []
    for k in range(NC):
        eids = eids_per_core[k]
        ne = len(eids)
        src_s = np.zeros(E_pad, np.int64)
        src_s[:ne] = src[eids]
        mask = np.zeros((128, E_pad), np.float32)
        maskT = np.zeros((128, E_pad), np.float32)
        tl = trg[eids] - k * TPC
        slots = np.arange(ne)
        mask[slots % 128, (slots // 128) * 128 + tl] = 1.0
        maskT[tl, slots] = 1.0
        # per-edge edge-feature rows, transposed (pure selection + layout)
        efe = np.zeros((E_pad, C), np.float32)
        efe[:ne] = ef[src[eids], trg[eids]]
        efT = np.ascontiguousarray(efe.T).reshape(2, 128, E_pad) \
            .transpose(1, 0, 2).copy()
        m = dict(common)
        m.update({
            "efT": efT,
            "xTl": np.ascontiguousarray(
                xT[:, k * NB_LOCAL:(k + 1) * NB_LOCAL]
            ).reshape(2, 128, NB_LOCAL).transpose(1, 0, 2).copy(),
            "isrc": _pack_idx(src_s),
            "mask": mask.astype(NPBF),
            "maskT": maskT.astype(NPBF),
        })
        in_maps.append(m)
    return in_maps, E_pad


# --------------------------------------------------------------------------
# device program
# --------------------------------------------------------------------------

def _build(E_pad: int, debug: bool = False):
    n_chunks = E_pad // 128
    supers = []  # (start_chunk, n_chunk) groups of up to 4 chunks
    c0 = 0
    while c0 < n_chunks:
        nj = min(4, n_chunks - c0)
        supers.append((c0, nj))
        c0 += nj
    nsup = len(supers)
    nc = bacc.Bacc("TRN2", target_bir_lowering=False, debug=False,
                   num_devices=NC, num_swdge_queues=4)

    efT_in = nc.dram_tensor("efT", [128, 2, E_pad], F32, kind="ExternalInput")
    xTf_in = nc.dram_tensor("xTf", [128, 2, N * B], BF16,
                            kind="ExternalInput")
    xTl_in = nc.dram_tensor("xTl", [128, 2, NB_LOCAL], BF16,
                            kind="ExternalInput")
    isrc_in = nc.dram_tensor("isrc", [128, E_pad // 16], I16,
                             kind="ExternalInput")
    mask_in = nc.dram_tensor("mask", [128, E_pad], BF16, kind="ExternalInput")
    maskT_in = nc.dram_tensor("maskT", [128, E_pad], BF16,
                              kind="ExternalInput")
    w_in = {}
    for nm, inner, dt in [("wn1cols", C, BF16), ("wn2cols", C, BF16),
                          ("wn1hd", C, F32), ("wn2hd", C, F32),
                          ("we1hd", C, F32), ("we2hd", C, F32),
                          ("hsel1", H, F32), ("hsel2", H, F32),
                          ("ablk1", 2 * H, F32), ("ablk2", 2 * H, F32)]:
        w_in[nm] = nc.dram_tensor(nm, [128, 2, inner], dt,
                                  kind="ExternalInput")
    y_out = nc.dram_tensor("y", [128, B * C], F32, kind="ExternalOutput")
    dbg = {}
    if debug:
        for nm, shape, dt in [("dbg_x1", [128, B * C], F32),
                              ("dbg_pe", [128, n_chunks, 2 * H], F32),
                              ("dbg_tbl", [N, ROW], BF16)]:
            dbg[nm] = nc.dram_tensor(nm, shape, dt, kind="ExternalOutput")

    from contextlib import ExitStack
    with tile.TileContext(nc) as tc:
        with ExitStack() as ctx:
            const = ctx.enter_context(tc.tile_pool(name="const", bufs=1))
            sb = ctx.enter_context(tc.tile_pool(name="sb", bufs=1))
            small = ctx.enter_context(tc.tile_pool(name="small", bufs=5))
            shpool = ctx.enter_context(tc.tile_pool(name="shpool", bufs=16))
            gpool = ctx.enter_context(tc.tile_pool(name="gpool", bufs=8))
            ps_small = ctx.enter_context(
                tc.tile_pool(name="ps_small", bufs=2, space="PSUM"))
            ps_pat = ctx.enter_context(
                tc.tile_pool(name="ps_pat", bufs=3, space="PSUM"))
            ps_out = ctx.enter_context(
                tc.tile_pool(name="ps_out", bufs=1, space="PSUM"))
            dram = ctx.enter_context(tc.tile_pool(name="dram", bufs=1,
                                                  space="DRAM"))

            ident = const.tile([128, 128], BF16)
            make_identity(nc, ident[:])

            # isrc first, then a tiny dummy gather: forces the GpSimd DMA
            # ucode library load NOW, while the DMA queues are still empty
            # (the lib swap waits for queue quiescence, so doing it after the
            # big const loads costs ~15us)
            isrc_t = const.tile([128, E_pad // 16], I16)
            nc.sync.dma_start(out=isrc_t[:], in_=isrc_in[:])
            dummy_g = const.tile([128, 1, 128], BF16, name="dummy_g",
                                 tag="dummy_g")
            nc.gpsimd.dma_gather(
                out_ap=dummy_g[:],
                in_ap=mask_in[:].rearrange("p (k e) -> (p k) e", e=128),
                idxs_ap=isrc_t[:, 0:8],
                num_idxs=128, num_idxs_reg=128,
                elem_size=128,
                single_packet=False, queue_num=0)

            # fused table rhs: [wncols | a_sb] per layer (bf16) - loaded first
            # so the table-1 build is not stuck behind the big DMAs
            wa1 = const.tile([128, 2, WA], BF16, name="wa1", tag="wa1")
            wa2 = const.tile([128, 2, WA], BF16, name="wa2", tag="wa2")
            nc.sync.dma_start(out=wa1[:, :, 0:C], in_=w_in["wn1cols"][:])
            w_sb = {}

            def load_w(names):
                for nm in names:
                    t = w_in[nm]
                    w_sb[nm] = const.tile([128, 2, t.shape[2]], t.dtype,
                                          name=f"w_{nm}", tag=f"w_{nm}")
                    nc.sync.dma_start(out=w_sb[nm][:], in_=t[:])

            # layer-1-critical loads first: table-1 build + gathers unblock
            load_w(["wn1hd", "ablk1", "we1hd", "we2hd", "hsel1", "hsel2"])
            # big pe/mask loads go through the Activation hwdge queue so they
            # overlap the sync-queue loads and do not extend the DMA-counting
            # semaphore the first gathers wait on
            efT_sb = const.tile([128, 2, E_pad], F32)
            nc.scalar.dma_start(out=efT_sb[:], in_=efT_in[:])
            mask_sb = const.tile([128, E_pad], BF16)
            nc.scalar.dma_start(out=mask_sb[:], in_=mask_in[:])
            xTf_sb = const.tile([128, 2, N * B], BF16)
            nc.sync.dma_start(out=xTf_sb[:], in_=xTf_in[:])
            xTl_sb = const.tile([128, 2, NB_LOCAL], BF16)
            nc.sync.dma_start(out=xTl_sb[:], in_=xTl_in[:])
            maskT_sb = const.tile([128, E_pad], BF16)
            nc.sync.dma_start(out=maskT_sb[:], in_=maskT_in[:])


            # ---- wesum[c, (layer h)] f32 from We row-sums per head, and
            # a_sb[c, (src/tgt h)] = Wn @ ablk (bf16) so x . a_sb = (x@Wn.T).a
            wesum_sb = const.tile([128, 2, 2 * H], F32)
            a1_sb = const.tile([128, 2, 2 * H], BF16)
            a2_sb = const.tile([128, 2, 2 * H], BF16)

            def a_prep(asb, wat, wnhd, ab):
                for ct in range(2):
                    pa0 = ps_small.tile([128, 2 * H], F32, space="PSUM",
                                        tag="ps", name="pa0")
                    for kh in range(2):
                        nc.tensor.matmul(
                            out=pa0[:],
                            lhsT=w_sb[wnhd][:, kh, ct * 128:(ct + 1) * 128],
                            rhs=w_sb[ab][:, kh, :],
                            start=(kh == 0), stop=(kh == 1))
                    nc.scalar.copy(out=asb[:, ct, :], in_=pa0[:])
                    nc.scalar.copy(out=wat[:, ct, C:WA], in_=pa0[:])

            a_prep(a1_sb, wa1, "wn1hd", "ablk1")

            # ---- table build: fused h|a matmul; rows are 4 per-b sections
            # of WA=264 bf16 units each, so every staging partition writes one
            # contiguous 528B run (fast DMA); 4 sub-DMAs run on 4 engines
            def build_table(lhsT_sb, wa, table, nblk):
                for t in range(nblk):
                    ph = ps_small.tile([128, WA], F32, space="PSUM", tag="ps",
                                       name="ph")
                    for ch in range(2):
                        nc.tensor.matmul(
                            out=ph[:],
                            lhsT=lhsT_sb[:, ch, t * 128:(t + 1) * 128],
                            rhs=wa[:, ch, :],
                            start=(ch == 0), stop=(ch == 1))
                    sh = shpool.tile([128, WA], BF16, tag="sh")
                    nc.scalar.copy(out=sh[:], in_=ph[:])
                    rows = slice(t * 32, (t + 1) * 32)
                    nc.sync.dma_start(
                        out=table[rows, 0:B * WA].rearrange(
                            "n (b s) -> n b s", b=B),
                        in_=sh[:])

            # per-b a_tgt for local targets: at_rhs [t(part), (b h)] bf16
            def build_at(lhsT_sb, asb):
                at_rhs = small.tile([128, B * H], BF16, tag="atr")
                for b in range(B):
                    pab = ps_small.tile([128, 2 * H], F32, space="PSUM",
                                        tag="ps", name="pab")
                    for ch in range(2):
                        lhsT_b = lhsT_sb[:, ch, :].rearrange(
                            "p (n b2) -> p b2 n", b2=B)[:, b, :]
                        nc.tensor.matmul(out=pab[:], lhsT=lhsT_b,
                                         rhs=asb[:, ch, :],
                                         start=(ch == 0), stop=(ch == 1))
                    nc.vector.tensor_copy(out=at_rhs[:, b * H:(b + 1) * H],
                                          in_=pab[:, H:2 * H])
                return at_rhs

            # ---- layer-1 table: fully replicated compute, private DRAM table
            table1 = dram.tile([N, ROW], BF16, tag="tbl1", name="tbl1")
            build_table(xTf_sb, wa1, table1, N // 32)
            at1 = build_at(xTl_sb, a1_sb)

            # deferred loads: layer-2-only inputs, issued after the table-1
            # writes so they do not delay the first gathers
            nc.sync.dma_start(out=wa2[:, :, 0:C], in_=w_in["wn2cols"][:])
            load_w(["wn2hd", "ablk2"])

            for ct in range(2):
                pw = ps_small.tile([128, 2 * H], F32, space="PSUM", tag="ps",
                                   name="pw")
                for lj, (wehd, hs) in enumerate(
                        [("we1hd", "hsel1"), ("we2hd", "hsel2")]):
                    for kh in range(2):
                        nc.tensor.matmul(
                            out=pw[:, lj * H:(lj + 1) * H],
                            lhsT=w_sb[wehd][:, kh, ct * 128:(ct + 1) * 128],
                            rhs=w_sb[hs][:, kh, :],
                            start=(kh == 0), stop=(kh == 1))
                nc.scalar.copy(out=wesum_sb[:, ct, :], in_=pw[:])

            # ---- pe emitter: pe_sb[e-chunk, (layer h)] f32, interleaved
            # with the layer-1 edge loop so L1 compute starts early
            pe_sb = sb.tile([128, n_chunks, 2 * H], F32)

            def emit_pe(si):
                s0, nj = supers[si]
                pp = ps_small.tile([128, 4, 2 * H], F32, space="PSUM",
                                   tag="ps", name="pp")
                for j in range(nj):
                    c = s0 + j
                    for ch in range(2):
                        nc.tensor.matmul(
                            out=pp[:, j, :],
                            lhsT=efT_sb[:, ch, c * 128:(c + 1) * 128],
                            rhs=wesum_sb[:, ch, :],
                            start=(ch == 0), stop=(ch == 1))
                nc.scalar.copy(out=pe_sb[:, s0:s0 + nj, :], in_=pp[:, 0:nj, :])

            # ---- edge loop for one layer (software-pipelined pat)
            PD = 3

            def edge_loop(table, at_rhs, layer, with_pe=False):
                out_p = ps_out.tile([128, B * WA], F32, space="PSUM",
                                    tag="out", name="out_p")
                Gs = []
                for si, (s0, nj) in enumerate(supers):
                    G = gpool.tile([128, 4, ROW], BF16, tag="G")
                    nc.gpsimd.dma_gather(
                        out_ap=G[:, 0:nj, :], in_ap=table[:],
                        idxs_ap=isrc_t[:, s0 * 8:(s0 + nj) * 8],
                        num_idxs=nj * 128, num_idxs_reg=nj * 128,
                        elem_size=ROW, single_packet=False,
                        queue_num=si % 4)
                    Gs.append(G)

                # pat + pe, gather-independent: spre = maskT@at + pe
                def emit_spre(si):
                    s0, nj = supers[si]
                    pat = ps_pat.tile([128, 4, B * H], F32, space="PSUM",
                                      tag="pat", name="pat")
                    for j in range(nj):
                        c = s0 + j
                        nc.tensor.matmul(
                            out=pat[:, j, :],
                            lhsT=maskT_sb[:, c * 128:(c + 1) * 128],
                            rhs=at_rhs[:], start=True, stop=True)
                    spre = small.tile([128, 4, B * H], F32, tag="spre")
                    nc.vector.tensor_tensor(
                        out=spre[:, 0:nj, :].rearrange(
                            "p c (b h) -> p c b h", b=B),
                        in0=pat[:, 0:nj, :].rearrange(
                            "p c (b h) -> p c b h", b=B),
                        in1=pe_sb[:, s0:s0 + nj, layer * H:(layer + 1) * H]
                            .unsqueeze(2).to_broadcast([128, nj, B, H]),
                        op=mybir.AluOpType.add)
                    return spre

                PE_PD = 4
                if with_pe:
                    for si in range(min(PE_PD, nsup)):
                        emit_pe(si)
                spres = [emit_spre(si) for si in range(min(PD, nsup))]
                for si, (s0, nj) in enumerate(supers):
                    if with_pe and si + PE_PD < nsup:
                        emit_pe(si + PE_PD)
                    if si + PD < nsup:
                        spres.append(emit_spre(si + PD))
                    G = Gs[si]
                    spre = spres[si]
                    s4 = small.tile([128, 4, B * H], F32, tag="s")
                    e4 = small.tile([128, 4, B * H], BF16, tag="e")
                    nc.vector.tensor_tensor(
                        out=s4[:, 0:nj, :].rearrange(
                            "p c (b h) -> p c b h", b=B),
                        in0=G[:, 0:nj, 0:B * WA].rearrange(
                            "p c (b s) -> p c b s", b=B)[:, :, :, C:C + H],
                        in1=spre[:, 0:nj, :].rearrange(
                            "p c (b h) -> p c b h", b=B),
                        op=mybir.AluOpType.add)
                    nc.scalar.activation(
                        out=s4[:, 0:nj, :], in_=s4[:, 0:nj, :],
                        func=mybir.ActivationFunctionType.Lrelu, alpha=0.2)
                    nc.scalar.activation(
                        out=e4[:, 0:nj, :], in_=s4[:, 0:nj, :],
                        func=mybir.ActivationFunctionType.Exp)
                    # exp into the spare 4 slots of each row section: the
                    # mask matmul then accumulates the softmax denominator
                    # into out_p for free (no separate den matmul)
                    nc.scalar.copy(
                        out=G[:, 0:nj, 0:B * WA].rearrange(
                            "p c (b s) -> p c b s", b=B)[:, :, :, C + H:WA],
                        in_=e4[:, 0:nj, :].rearrange(
                            "p c (b h) -> p c b h", b=B))
                    # h *= exp  (dual-pumped: inner dim h contiguous)
                    for j in range(nj):
                        hv = G[:, j, 0:B * WA].rearrange(
                            "p (b s) -> p b s", b=B)[:, :, 0:C].rearrange(
                            "p b (d h) -> p b d h", d=D)
                        nc.vector.tensor_tensor(
                            out=hv, in0=hv,
                            in1=e4[:, j, :].rearrange("p (b h) -> p b h", b=B)
                                .unsqueeze(2).to_broadcast([128, B, D, H]),
                            op=mybir.AluOpType.mult)
                    # scatter: out += mask.T @ (exp*h) ; den += mask.T @ exp
                    for j in range(nj):
                        c = s0 + j
                        mk = mask_sb[:, c * 128:(c + 1) * 128]
                        first, last = (c == 0), (c == n_chunks - 1)
                        nc.tensor.matmul(out=out_p[:, 0:512], lhsT=mk,
                                         rhs=G[:, j, 0:512],
                                         start=first, stop=last)
                        nc.tensor.matmul(out=out_p[:, 512:1024], lhsT=mk,
                                         rhs=G[:, j, 512:1024],
                                         start=first, stop=last)
                        nc.tensor.matmul(out=out_p[:, 1024:B * WA], lhsT=mk,
                                         rhs=G[:, j, 1024:B * WA],
                                         start=first, stop=last)
                dsb = small.tile([128, B * H], F32, tag="d")
                nc.vector.tensor_scalar_add(
                    dsb[:].rearrange("p (b h) -> p b h", b=B),
                    out_p[:].rearrange(
                        "p (b s) -> p b s", b=B)[:, :, C + H:WA],
                    1e-16)
                rec = small.tile([128, B * H], F32, tag="r")
                nc.vector.reciprocal(rec[:], dsb[:])
                # divide + un-permute (b,d,h) -> (b,h,d)
                xo = sb.tile([128, B * C], F32, tag=f"xo{layer}",
                             name=f"xo{layer}")
                nc.vector.tensor_tensor(
                    out=xo[:].rearrange("p (b h d) -> p b h d", b=B, h=H),
                    in0=out_p[:].rearrange(
                        "p (b s) -> p b s", b=B)[:, :, 0:C].rearrange(
                        "p b (d h) -> p b h d", d=D),
                    in1=rec[:].rearrange("p (b h) -> p b h", b=B)
                        .unsqueeze(3).to_broadcast([128, B, H, D]),
                    op=mybir.AluOpType.mult)
                return xo

            x1 = edge_loop(table1, at1, 0, with_pe=True)
            a_prep(a2_sb, wa2, "wn2hd", "ablk2")
            if debug:
                nc.sync.dma_start(out=dbg["dbg_pe"][:], in_=pe_sb[:])

            # tiny warm-up collective: first collective pays a ~10-20us
            # start-up cost; run it here (after the L1 gather desc-gen on the
            # gpsimd queue, long before AG2 needs the CC path warm)
            warm_in = dram.tile([TPC, ROW], BF16, name="warm_in",
                                tag="warm_in")
            warm_out = dram.tile([N, ROW], BF16, addr_space="Shared",
                                 name="warm_out", tag="warm_out")
            nc.gpsimd.collective_compute(
                "AllGather", mybir.AluOpType.bypass,
                replica_groups=[list(range(NC))],
                ins=[warm_in.opt()], outs=[warm_out.opt()])

            if debug:
                nc.sync.dma_start(out=dbg["dbg_x1"][:], in_=x1[:])
                nc.sync.dma_start(out=dbg["dbg_tbl"][:], in_=table1[:])

            # ---- x1 -> bf16 -> transpose -> layer-2 table + AllGather
            x1b = sb.tile([128, B * C], BF16)
            nc.vector.tensor_copy(out=x1b[:], in_=x1[:])
            x1T = sb.tile([128, 2, NB_LOCAL], BF16)
            for b in range(B):
                for ch in range(2):
                    pt = ps_small.tile([128, 128], BF16, space="PSUM",
                                       tag="ps", name="pt")
                    nc.tensor.transpose(
                        out=pt[:],
                        in_=x1b[:, b * C + ch * 128: b * C + (ch + 1) * 128],
                        identity=ident[:])
                    nc.scalar.copy(
                        out=x1T[:, ch, :].rearrange(
                            "p (n b2) -> p n b2", b2=B)[:, :, b],
                        in_=pt[:])

            ag_in2 = dram.tile([TPC, ROW], BF16, tag="agin2", name="agin2")
            table2 = dram.tile([N, ROW], BF16, addr_space="Shared",
                               tag="tbl2", name="tbl2")
            build_table(x1T, wa2, ag_in2, TPC // 32)
            at2 = build_at(x1T, a2_sb)
            nc.gpsimd.collective_compute(
                "AllGather", mybir.AluOpType.bypass,
                replica_groups=[list(range(NC))],
                ins=[ag_in2.opt()], outs=[table2.opt()])

            x2 = edge_loop(table2, at2, 1)
            nc.sync.dma_start(out=y_out[:], in_=x2[:])

    nc.compile()
    return nc


_CACHE: dict = {}


def _get_program(E_pad: int, debug: bool = False):
    key = (E_pad, debug)
    if key not in _CACHE:
        _CACHE[key] = _build(E_pad, debug)
    return _CACHE[key]


def kernel(debug=False, trace=False, **inputs):
    in_maps, E_pad = _prep(**inputs)
    nc = _get_program(E_pad, debug)
    res = run_bass_kernel_spmd(nc, in_maps, core_ids=list(range(NC)),
                               trace=trace)
    y = np.concatenate([res.results[k]["y"] for k in range(NC)], axis=0)
    out = y.reshape(N, B, C)
    if debug or trace:
        return out, res
    return out


# revision 32
# speedup vs baseline: 1.1623x; 1.1623x over previous
"""Trainium2 Bass kernel for a 2-layer GAT (nn_GAT_70909910057105).

Strategy (8 NeuronCores, SPMD):
  - Core k owns target nodes [128k, 128k+128). Edges bucketed by trg//128 on
    the host (integer/layout-only preprocessing), padded to a uniform E_pad.
  - Per-edge edge_feature rows are staged host-side (pure row selection) as a
    transposed f32 input efT [C, E_pad], so pe = efT.T @ wesum is a plain
    matmul burst on device - no edge-feature gather, no PE transposes. The
    burst runs while the CC barrier + table-1 AllGather complete.
  - Node tables ([1024 bf16 h (b,d,h layout) | 16 f32 a_src | pad] rows) are
    exchanged with one AllGather per layer (a tiny warm-up collective absorbs
    the CC cold-start) and fetched per-edge with dma_gather on 4 SWDGE queues.
  - segment_sum is a PSUM-accumulated bf16 matmul with host-built one-hot
    masks; per-edge target alphas come from software-pipelined maskT @ at
    matmuls, summed with pe before the gather lands.
  - The h row layout (b, d, h) (host-permuted Wn columns) keeps the exp*h
    multiply inner-contiguous so the DVE runs it dual-pumped.
"""
import sys

for _p in ("/opt/trn_rl_repo", "/root/.axon_site/_ro/trn_rl_repo"):
    if _p not in sys.path:
        sys.path.insert(0, _p)

import numpy as np
import ml_dtypes
import concourse.bass as bass
import concourse.bacc as bacc
import concourse.tile as tile
from concourse import mybir
from concourse.bass_utils import run_bass_kernel_spmd
from concourse.masks import make_identity

F32 = mybir.dt.float32
BF16 = mybir.dt.bfloat16
I16 = mybir.dt.int16
NPBF = ml_dtypes.bfloat16

N, B, C, H, D = 1024, 4, 256, 4, 64
E = 32768
NC = 8
TPC = N // NC           # target nodes per core = 128
ROW = 1152              # bf16 elems: 1024 h | 32 (16 f32 a_src) | 96 pad
AS_OFF = 1024           # bf16-elem offset of the a_src f32 region
NB_LOCAL = TPC * B      # 512 local (node, batch) columns
WA = C + 2 * H          # fused table rhs width: wncols ++ a_sb

# permutation: table h column j = d*4 + h  <->  model channel h*64 + d
_PERM = np.array([(j % H) * D + j // H for j in range(C)])


# --------------------------------------------------------------------------
# host-side preprocessing (integer / layout ops only)
# --------------------------------------------------------------------------

def _pack_idx(vals: np.ndarray) -> np.ndarray:
    n = vals.shape[0]
    assert n % 16 == 0
    blk = vals.astype(np.int16).reshape(n // 16, 16).T
    return np.ascontiguousarray(np.tile(blk, (8, 1)))


def _prep(x, edge_features, src_idx, trg_idx,
          Wn1, We1, a_src1, a_tgt1, a_edge1,
          Wn2, We2, a_src2, a_tgt2, a_edge2):
    src = np.asarray(src_idx).astype(np.int64)
    trg = np.asarray(trg_idx).astype(np.int64)
    x = np.asarray(x, dtype=np.float32)
    ef = np.asarray(edge_features, dtype=np.float32)

    # edges per core, split into two waves by src%128 (<64 / >=64): wave-0
    # edges only need the first half-AllGather of the layer-2 table
    def pad128(n):
        return ((n + 127) // 128) * 128

    waves_per_core = []
    for k in range(NC):
        eids = np.nonzero((trg // TPC) == k)[0]
        w0 = eids[(src[eids] % TPC) < (TPC // 2)]
        w1 = eids[(src[eids] % TPC) >= (TPC // 2)]
        waves_per_core.append((w0, w1))
    W0_pad = max(pad128(len(w0)) for w0, _ in waves_per_core)
    W1_pad = max(pad128(len(w1)) for _, w1 in waves_per_core)
    E_pad = W0_pad + W1_pad

    xf = x.reshape(N * B, C)
    xT = np.ascontiguousarray(xf.T).astype(NPBF)

    def sb3(w, inner):
        return np.ascontiguousarray(w.reshape(2, 128, inner).transpose(1, 0, 2))

    def hsel(a_e):
        m = np.zeros((C, H), np.float32)
        for h in range(H):
            m[h * D:(h + 1) * D, h] = np.float32(a_e[h])
        return sb3(m, H)

    def ablk(a_s, a_t):
        m = np.zeros((C, 2 * H), np.float32)
        for h in range(H):
            m[h * D:(h + 1) * D, h] = np.asarray(a_s)[h]
            m[h * D:(h + 1) * D, H + h] = np.asarray(a_t)[h]
        return sb3(m, 2 * H)

    def wncols_p(Wn):
        wt = np.ascontiguousarray(np.asarray(Wn, np.float32).T)  # [c, c']
        return sb3(wt[:, _PERM].astype(NPBF), C)

    common = {
        "xTf": np.ascontiguousarray(xT).reshape(2, 128, N * B)
                 .transpose(1, 0, 2).copy(),
        "wn1cols": wncols_p(Wn1),
        "wn2cols": wncols_p(Wn2),
        "wn1hd": sb3(np.asarray(Wn1, np.float32), C),
        "wn2hd": sb3(np.asarray(Wn2, np.float32), C),
        "we1hd": sb3(np.asarray(We1, np.float32), C),
        "we2hd": sb3(np.asarray(We2, np.float32), C),
        "hsel1": hsel(np.asarray(a_edge1)),
        "hsel2": hsel(np.asarray(a_edge2)),
        "ablk1": ablk(a_src1, a_tgt1),
        "ablk2": ablk(a_src2, a_tgt2),
    }

    in_maps = []
    for k in range(NC):
        w0, w1 = waves_per_core[k]
        src_s = np.zeros(E_pad, np.int64)    # L1 idx: global node row
        src2 = np.zeros(E_pad, np.int64)     # L2 idx: half-table row
        mask = np.zeros((128, E_pad), np.float32)
        maskT = np.zeros((128, E_pad), np.float32)
        efe = np.zeros((E_pad, C), np.float32)
        for base, eids in ((0, w0), (W0_pad, w1)):
            ne = len(eids)
            s = src[eids]
            src_s[base:base + ne] = s
            src2[base:base + ne] = (s // TPC) * (TPC // 2) + (s % TPC) % (
                TPC // 2)
            tl = trg[eids] - k * TPC
            slots = base + np.arange(ne)
            mask[slots % 128, (slots // 128) * 128 + tl] = 1.0
            maskT[tl, slots] = 1.0
            efe[base:base + ne] = ef[s, trg[eids]]
        efT = np.ascontiguousarray(efe.T).reshape(2, 128, E_pad) \
            .transpose(1, 0, 2).copy()
        m = dict(common)
        m.update({
            "efT": efT,
            "xTl": np.ascontiguousarray(
                xT[:, k * NB_LOCAL:(k + 1) * NB_LOCAL]
            ).reshape(2, 128, NB_LOCAL).transpose(1, 0, 2).copy(),
            "isrc": _pack_idx(src_s),
            "isrc2": _pack_idx(src2),
            "mask": mask.astype(NPBF),
            "maskT": maskT.astype(NPBF),
        })
        in_maps.append(m)
    return in_maps, W0_pad, W1_pad


# --------------------------------------------------------------------------
# device program
# --------------------------------------------------------------------------

def _build(W0_pad: int, W1_pad: int, debug: bool = False):
    E_pad = W0_pad + W1_pad
    n_chunks = E_pad // 128
    supers = []  # (start_chunk, n_chunk, wave) groups of up to 4 chunks
    for lo, hi, wv in ((0, W0_pad // 128, 0),
                       (W0_pad // 128, n_chunks, 1)):
        c0 = lo
        while c0 < hi:
            nj = min(4, hi - c0)
            supers.append((c0, nj, wv))
            c0 += nj
    nsup = len(supers)
    nc = bacc.Bacc("TRN2", target_bir_lowering=False, debug=False,
                   num_devices=NC, num_swdge_queues=4)

    efT_in = nc.dram_tensor("efT", [128, 2, E_pad], F32, kind="ExternalInput")
    xTf_in = nc.dram_tensor("xTf", [128, 2, N * B], BF16,
                            kind="ExternalInput")
    xTl_in = nc.dram_tensor("xTl", [128, 2, NB_LOCAL], BF16,
                            kind="ExternalInput")
    isrc_in = nc.dram_tensor("isrc", [128, E_pad // 16], I16,
                             kind="ExternalInput")
    isrc2_in = nc.dram_tensor("isrc2", [128, E_pad // 16], I16,
                              kind="ExternalInput")
    mask_in = nc.dram_tensor("mask", [128, E_pad], BF16, kind="ExternalInput")
    maskT_in = nc.dram_tensor("maskT", [128, E_pad], BF16,
                              kind="ExternalInput")
    w_in = {}
    for nm, inner, dt in [("wn1cols", C, BF16), ("wn2cols", C, BF16),
                          ("wn1hd", C, F32), ("wn2hd", C, F32),
                          ("we1hd", C, F32), ("we2hd", C, F32),
                          ("hsel1", H, F32), ("hsel2", H, F32),
                          ("ablk1", 2 * H, F32), ("ablk2", 2 * H, F32)]:
        w_in[nm] = nc.dram_tensor(nm, [128, 2, inner], dt,
                                  kind="ExternalInput")
    y_out = nc.dram_tensor("y", [128, B * C], F32, kind="ExternalOutput")
    dbg = {}
    if debug:
        for nm, shape, dt in [("dbg_x1", [128, B * C], F32),
                              ("dbg_pe", [128, n_chunks, 2 * H], F32),
                              ("dbg_tbl", [N, ROW], BF16)]:
            dbg[nm] = nc.dram_tensor(nm, shape, dt, kind="ExternalOutput")

    from contextlib import ExitStack
    with tile.TileContext(nc) as tc:
        with ExitStack() as ctx:
            const = ctx.enter_context(tc.tile_pool(name="const", bufs=1))
            sb = ctx.enter_context(tc.tile_pool(name="sb", bufs=1))
            small = ctx.enter_context(tc.tile_pool(name="small", bufs=5))
            shpool = ctx.enter_context(tc.tile_pool(name="shpool", bufs=16))
            gpool = ctx.enter_context(tc.tile_pool(name="gpool", bufs=8))
            ps_small = ctx.enter_context(
                tc.tile_pool(name="ps_small", bufs=2, space="PSUM"))
            ps_pat = ctx.enter_context(
                tc.tile_pool(name="ps_pat", bufs=3, space="PSUM"))
            ps_out = ctx.enter_context(
                tc.tile_pool(name="ps_out", bufs=1, space="PSUM"))
            dram = ctx.enter_context(tc.tile_pool(name="dram", bufs=1,
                                                  space="DRAM"))

            ident = const.tile([128, 128], BF16)
            make_identity(nc, ident[:])

            # isrc first, then a tiny dummy gather: forces the GpSimd DMA
            # ucode library load NOW, while the DMA queues are still empty
            # (the lib swap waits for queue quiescence, so doing it after the
            # big const loads costs ~15us)
            isrc_t = const.tile([128, E_pad // 16], I16)
            nc.sync.dma_start(out=isrc_t[:], in_=isrc_in[:])
            isrc2_t = const.tile([128, E_pad // 16], I16)
            nc.sync.dma_start(out=isrc2_t[:], in_=isrc2_in[:])
            dummy_g = const.tile([128, 1, 128], BF16, name="dummy_g",
                                 tag="dummy_g")
            nc.gpsimd.dma_gather(
                out_ap=dummy_g[:],
                in_ap=mask_in[:].rearrange("p (k e) -> (p k) e", e=128),
                idxs_ap=isrc_t[:, 0:8],
                num_idxs=128, num_idxs_reg=128,
                elem_size=128,
                single_packet=False, queue_num=0)

            # fused table rhs: [wncols | a_sb] per layer (bf16) - loaded first
            # so the table-1 build is not stuck behind the big DMAs
            wa1 = const.tile([128, 2, WA], BF16, name="wa1", tag="wa1")
            wa2 = const.tile([128, 2, WA], BF16, name="wa2", tag="wa2")
            nc.sync.dma_start(out=wa1[:, :, 0:C], in_=w_in["wn1cols"][:])
            w_sb = {}

            def load_w(names):
                for nm in names:
                    t = w_in[nm]
                    w_sb[nm] = const.tile([128, 2, t.shape[2]], t.dtype,
                                          name=f"w_{nm}", tag=f"w_{nm}")
                    nc.sync.dma_start(out=w_sb[nm][:], in_=t[:])

            # layer-1-critical loads first: table-1 build + gathers unblock
            load_w(["wn1hd", "ablk1", "we1hd", "we2hd", "hsel1", "hsel2"])
            # big pe/mask loads go through the Activation hwdge queue so they
            # overlap the sync-queue loads and do not extend the DMA-counting
            # semaphore the first gathers wait on
            efT_sb = const.tile([128, 2, E_pad], F32)
            nc.scalar.dma_start(out=efT_sb[:], in_=efT_in[:])
            mask_sb = const.tile([128, E_pad], BF16)
            nc.scalar.dma_start(out=mask_sb[:], in_=mask_in[:])
            xTf_sb = const.tile([128, 2, N * B], BF16)
            nc.sync.dma_start(out=xTf_sb[:], in_=xTf_in[:])
            xTl_sb = const.tile([128, 2, NB_LOCAL], BF16)
            nc.sync.dma_start(out=xTl_sb[:], in_=xTl_in[:])
            maskT_sb = const.tile([128, E_pad], BF16)
            nc.sync.dma_start(out=maskT_sb[:], in_=maskT_in[:])


            # ---- wesum[c, (layer h)] f32 from We row-sums per head, and
            # a_sb[c, (src/tgt h)] = Wn @ ablk (bf16) so x . a_sb = (x@Wn.T).a
            wesum_sb = const.tile([128, 2, 2 * H], F32)
            a1_sb = const.tile([128, 2, 2 * H], BF16)
            a2_sb = const.tile([128, 2, 2 * H], BF16)

            def a_prep(asb, wat, wnhd, ab):
                for ct in range(2):
                    pa0 = ps_small.tile([128, 2 * H], F32, space="PSUM",
                                        tag="ps", name="pa0")
                    for kh in range(2):
                        nc.tensor.matmul(
                            out=pa0[:],
                            lhsT=w_sb[wnhd][:, kh, ct * 128:(ct + 1) * 128],
                            rhs=w_sb[ab][:, kh, :],
                            start=(kh == 0), stop=(kh == 1))
                    nc.scalar.copy(out=asb[:, ct, :], in_=pa0[:])
                    nc.scalar.copy(out=wat[:, ct, C:WA], in_=pa0[:])

            a_prep(a1_sb, wa1, "wn1hd", "ablk1")

            # ---- table build: fused h|a matmul; rows are 4 per-b sections
            # of WA=264 bf16 units each, so every staging partition writes one
            # contiguous 528B run (fast DMA); 4 sub-DMAs run on 4 engines
            def build_table(lhsT_sb, wa, table, nblk):
                for t in range(nblk):
                    ph = ps_small.tile([128, WA], F32, space="PSUM", tag="ps",
                                       name="ph")
                    for ch in range(2):
                        nc.tensor.matmul(
                            out=ph[:],
                            lhsT=lhsT_sb[:, ch, t * 128:(t + 1) * 128],
                            rhs=wa[:, ch, :],
                            start=(ch == 0), stop=(ch == 1))
                    sh = shpool.tile([128, WA], BF16, tag="sh")
                    nc.scalar.copy(out=sh[:], in_=ph[:])
                    rows = slice(t * 32, (t + 1) * 32)
                    nc.sync.dma_start(
                        out=table[rows, 0:B * WA].rearrange(
                            "n (b s) -> n b s", b=B),
                        in_=sh[:])

            # per-b a_tgt for local targets: at_rhs [t(part), (b h)] bf16
            def build_at(lhsT_sb, asb):
                at_rhs = small.tile([128, B * H], BF16, tag="atr")
                for b in range(B):
                    pab = ps_small.tile([128, 2 * H], F32, space="PSUM",
                                        tag="ps", name="pab")
                    for ch in range(2):
                        lhsT_b = lhsT_sb[:, ch, :].rearrange(
                            "p (n b2) -> p b2 n", b2=B)[:, b, :]
                        nc.tensor.matmul(out=pab[:], lhsT=lhsT_b,
                                         rhs=asb[:, ch, :],
                                         start=(ch == 0), stop=(ch == 1))
                    nc.vector.tensor_copy(out=at_rhs[:, b * H:(b + 1) * H],
                                          in_=pab[:, H:2 * H])
                return at_rhs

            # ---- layer-1 table: fully replicated compute, private DRAM table
            table1 = dram.tile([N, ROW], BF16, tag="tbl1", name="tbl1")
            build_table(xTf_sb, wa1, table1, N // 32)
            at1 = build_at(xTl_sb, a1_sb)

            # deferred loads: layer-2-only inputs, issued after the table-1
            # writes so they do not delay the first gathers
            nc.sync.dma_start(out=wa2[:, :, 0:C], in_=w_in["wn2cols"][:])
            load_w(["wn2hd", "ablk2"])

            for ct in range(2):
                pw = ps_small.tile([128, 2 * H], F32, space="PSUM", tag="ps",
                                   name="pw")
                for lj, (wehd, hs) in enumerate(
                        [("we1hd", "hsel1"), ("we2hd", "hsel2")]):
                    for kh in range(2):
                        nc.tensor.matmul(
                            out=pw[:, lj * H:(lj + 1) * H],
                            lhsT=w_sb[wehd][:, kh, ct * 128:(ct + 1) * 128],
                            rhs=w_sb[hs][:, kh, :],
                            start=(kh == 0), stop=(kh == 1))
                nc.scalar.copy(out=wesum_sb[:, ct, :], in_=pw[:])

            # ---- pe emitter: pe_sb[e-chunk, (layer h)] f32, interleaved
            # with the layer-1 edge loop so L1 compute starts early
            pe_sb = sb.tile([128, n_chunks, 2 * H], F32)

            def emit_pe(si):
                s0, nj, _ = supers[si]
                pp = ps_small.tile([128, 4, 2 * H], F32, space="PSUM",
                                   tag="ps", name="pp")
                for j in range(nj):
                    c = s0 + j
                    for ch in range(2):
                        nc.tensor.matmul(
                            out=pp[:, j, :],
                            lhsT=efT_sb[:, ch, c * 128:(c + 1) * 128],
                            rhs=wesum_sb[:, ch, :],
                            start=(ch == 0), stop=(ch == 1))
                nc.scalar.copy(out=pe_sb[:, s0:s0 + nj, :], in_=pp[:, 0:nj, :])

            # ---- edge loop for one layer (software-pipelined pat)
            PD = 3

            def edge_loop(tables, idx_t, at_rhs, layer, with_pe=False):
                out_p = ps_out.tile([128, B * WA], F32, space="PSUM",
                                    tag="out", name="out_p")
                Gs = []
                for si, (s0, nj, wv) in enumerate(supers):
                    G = gpool.tile([128, 4, ROW], BF16, tag="G")
                    nc.gpsimd.dma_gather(
                        out_ap=G[:, 0:nj, :], in_ap=tables[wv][:],
                        idxs_ap=idx_t[:, s0 * 8:(s0 + nj) * 8],
                        num_idxs=nj * 128, num_idxs_reg=nj * 128,
                        elem_size=ROW, single_packet=False,
                        queue_num=si % 4)
                    Gs.append(G)

                # pat + pe, gather-independent: spre = maskT@at + pe
                def emit_spre(si):
                    s0, nj, _ = supers[si]
                    pat = ps_pat.tile([128, 4, B * H], F32, space="PSUM",
                                      tag="pat", name="pat")
                    for j in range(nj):
                        c = s0 + j
                        nc.tensor.matmul(
                            out=pat[:, j, :],
                            lhsT=maskT_sb[:, c * 128:(c + 1) * 128],
                            rhs=at_rhs[:], start=True, stop=True)
                    spre = small.tile([128, 4, B * H], F32, tag="spre")
                    nc.vector.tensor_tensor(
                        out=spre[:, 0:nj, :].rearrange(
                            "p c (b h) -> p c b h", b=B),
                        in0=pat[:, 0:nj, :].rearrange(
                            "p c (b h) -> p c b h", b=B),
                        in1=pe_sb[:, s0:s0 + nj, layer * H:(layer + 1) * H]
                            .unsqueeze(2).to_broadcast([128, nj, B, H]),
                        op=mybir.AluOpType.add)
                    return spre

                PE_PD = 4
                if with_pe:
                    for si in range(min(PE_PD, nsup)):
                        emit_pe(si)
                spres = [emit_spre(si) for si in range(min(PD, nsup))]
                for si, (s0, nj, wv) in enumerate(supers):
                    if with_pe and si + PE_PD < nsup:
                        emit_pe(si + PE_PD)
                    if si + PD < nsup:
                        spres.append(emit_spre(si + PD))
                    G = Gs[si]
                    spre = spres[si]
                    s4 = small.tile([128, 4, B * H], F32, tag="s")
                    e4 = small.tile([128, 4, B * H], BF16, tag="e")
                    nc.vector.tensor_tensor(
                        out=s4[:, 0:nj, :].rearrange(
                            "p c (b h) -> p c b h", b=B),
                        in0=G[:, 0:nj, 0:B * WA].rearrange(
                            "p c (b s) -> p c b s", b=B)[:, :, :, C:C + H],
                        in1=spre[:, 0:nj, :].rearrange(
                            "p c (b h) -> p c b h", b=B),
                        op=mybir.AluOpType.add)
                    nc.scalar.activation(
                        out=s4[:, 0:nj, :], in_=s4[:, 0:nj, :],
                        func=mybir.ActivationFunctionType.Lrelu, alpha=0.2)
                    nc.scalar.activation(
                        out=e4[:, 0:nj, :], in_=s4[:, 0:nj, :],
                        func=mybir.ActivationFunctionType.Exp)
                    # exp into the spare 4 slots of each row section: the
                    # mask matmul then accumulates the softmax denominator
                    # into out_p for free (no separate den matmul)
                    nc.scalar.copy(
                        out=G[:, 0:nj, 0:B * WA].rearrange(
                            "p c (b s) -> p c b s", b=B)[:, :, :, C + H:WA],
                        in_=e4[:, 0:nj, :].rearrange(
                            "p c (b h) -> p c b h", b=B))
                    # h *= exp  (dual-pumped: inner dim h contiguous)
                    for j in range(nj):
                        hv = G[:, j, 0:B * WA].rearrange(
                            "p (b s) -> p b s", b=B)[:, :, 0:C].rearrange(
                            "p b (d h) -> p b d h", d=D)
                        nc.vector.tensor_tensor(
                            out=hv, in0=hv,
                            in1=e4[:, j, :].rearrange("p (b h) -> p b h", b=B)
                                .unsqueeze(2).to_broadcast([128, B, D, H]),
                            op=mybir.AluOpType.mult)
                    # scatter: out += mask.T @ (exp*h) ; den += mask.T @ exp
                    for j in range(nj):
                        c = s0 + j
                        mk = mask_sb[:, c * 128:(c + 1) * 128]
                        first, last = (c == 0), (c == n_chunks - 1)
                        nc.tensor.matmul(out=out_p[:, 0:512], lhsT=mk,
                                         rhs=G[:, j, 0:512],
                                         start=first, stop=last)
                        nc.tensor.matmul(out=out_p[:, 512:1024], lhsT=mk,
                                         rhs=G[:, j, 512:1024],
                                         start=first, stop=last)
                        nc.tensor.matmul(out=out_p[:, 1024:B * WA], lhsT=mk,
                                         rhs=G[:, j, 1024:B * WA],
                                         start=first, stop=last)
                dsb = small.tile([128, B * H], F32, tag="d")
                nc.vector.tensor_scalar_add(
                    dsb[:].rearrange("p (b h) -> p b h", b=B),
                    out_p[:].rearrange(
                        "p (b s) -> p b s", b=B)[:, :, C + H:WA],
                    1e-16)
                rec = small.tile([128, B * H], F32, tag="r")
                nc.vector.reciprocal(rec[:], dsb[:])
                # divide + un-permute (b,d,h) -> (b,h,d)
                xo = sb.tile([128, B * C], F32, tag=f"xo{layer}",
                             name=f"xo{layer}")
                nc.vector.tensor_tensor(
                    out=xo[:].rearrange("p (b h d) -> p b h d", b=B, h=H),
                    in0=out_p[:].rearrange(
                        "p (b s) -> p b s", b=B)[:, :, 0:C].rearrange(
                        "p b (d h) -> p b h d", d=D),
                    in1=rec[:].rearrange("p (b h) -> p b h", b=B)
                        .unsqueeze(3).to_broadcast([128, B, H, D]),
                    op=mybir.AluOpType.mult)
                return xo

            x1 = edge_loop((table1, table1), isrc_t, at1, 0, with_pe=True)
            a_prep(a2_sb, wa2, "wn2hd", "ablk2")
            if debug:
                nc.sync.dma_start(out=dbg["dbg_pe"][:], in_=pe_sb[:])

            # tiny warm-up collective: first collective pays a ~10-20us
            # start-up cost; run it here (after the L1 gather desc-gen on the
            # gpsimd queue, long before AG2 needs the CC path warm)
            warm_in = dram.tile([16, 16], F32, name="warm_in",
                                tag="warm_in")
            warm_out = dram.tile([128, 16], F32, addr_space="Shared",
                                 name="warm_out", tag="warm_out")
            nc.gpsimd.collective_compute(
                "AllGather", mybir.AluOpType.bypass,
                replica_groups=[list(range(NC))],
                ins=[warm_in.opt()], outs=[warm_out.opt()])

            if debug:
                nc.sync.dma_start(out=dbg["dbg_x1"][:], in_=x1[:])
                nc.sync.dma_start(out=dbg["dbg_tbl"][:], in_=table1[:])

            # ---- x1 -> bf16 -> transpose -> layer-2 table + AllGather
            x1b = sb.tile([128, B * C], BF16)
            nc.vector.tensor_copy(out=x1b[:], in_=x1[:])
            x1T = sb.tile([128, 2, NB_LOCAL], BF16)
            for b in range(B):
                for ch in range(2):
                    pt = ps_small.tile([128, 128], BF16, space="PSUM",
                                       tag="ps", name="pt")
                    nc.tensor.transpose(
                        out=pt[:],
                        in_=x1b[:, b * C + ch * 128: b * C + (ch + 1) * 128],
                        identity=ident[:])
                    nc.scalar.copy(
                        out=x1T[:, ch, :].rearrange(
                            "p (n b2) -> p n b2", b2=B)[:, :, b],
                        in_=pt[:])

            # layer-2 table exchanged in two half AllGathers: wave-0 edges
            # (src%128 < 64) gather from tblA as soon as the first half lands
            ag_in2 = dram.tile([TPC, ROW], BF16, tag="agin2", name="agin2")
            tblA = dram.tile([N // 2, ROW], BF16, addr_space="Shared",
                             tag="tblA", name="tblA")
            tblB = dram.tile([N // 2, ROW], BF16, addr_space="Shared",
                             tag="tblB", name="tblB")
            build_table(x1T, wa2, ag_in2, TPC // 32)
            at2 = build_at(x1T, a2_sb)
            nc.gpsimd.collective_compute(
                "AllGather", mybir.AluOpType.bypass,
                replica_groups=[list(range(NC))],
                ins=[ag_in2[0:TPC // 2, :].opt()], outs=[tblA.opt()])
            nc.gpsimd.collective_compute(
                "AllGather", mybir.AluOpType.bypass,
                replica_groups=[list(range(NC))],
                ins=[ag_in2[TPC // 2:TPC, :].opt()], outs=[tblB.opt()])

            x2 = edge_loop((tblA, tblB), isrc2_t, at2, 1)
            nc.sync.dma_start(out=y_out[:], in_=x2[:])

    nc.compile()
    return nc


_CACHE: dict = {}


def _get_program(W0_pad: int, W1_pad: int, debug: bool = False):
    key = (W0_pad, W1_pad, debug)
    if key not in _CACHE:
        _CACHE[key] = _build(W0_pad, W1_pad, debug)
    return _CACHE[key]


def kernel(debug=False, trace=False, **inputs):
    in_maps, W0_pad, W1_pad = _prep(**inputs)
    nc = _get_program(W0_pad, W1_pad, debug)
    res = run_bass_kernel_spmd(nc, in_maps, core_ids=list(range(NC)),
                               trace=trace)
    y = np.concatenate([res.results[k]["y"] for k in range(NC)], axis=0)
    out = y.reshape(N, B, C)
    if debug or trace:
        return out, res
    return out
